# revision 15
# baseline (speedup 1.0000x reference)
"""Trainium2 Bass kernel for GroupNorm(32) + single-head attention block.

Reference computation (per batch element b of 4, c=256, h=w=64, n=h*w=4096):
    xn = GroupNorm(32)(x) * gamma + beta
    q, k, v = split(W_qkv @ xn + b_qkv)          # 1x1 convs == channel matmuls
    S = (q^T k) / sqrt(c);  A = softmax(S);  o = A v
    out = W_out @ o + b_out + x

Sharding: 8 cores = 4 batch elements x 2 query-row halves. Each core gets its
full batch element (for GN stats and K/V) plus its half of the rows (for Q and
the residual). All cores run one identical SPMD graph; per-core behaviour
differs only through the data passed in. No collectives.

On-chip layouts (per core):
    x_full -> xf [128, 2, 4096] f32 (c on partitions)
    xn (bf16), K [c, n], Q [c, i] via matmuls with W^T; V^T [n, c] directly
    from matmul(lhsT=xn_chunk, rhs=WvT) so no transpose of V is needed.
    Scores are computed transposed (S^T[j, i] = K^T Q), softmax runs without
    max-subtraction (|S| <= ~7 for this distribution), the row sums come for
    free from a ones-column appended to V, and the PV result is transposed
    back with PE-transposes before the output projection.
"""

import numpy as np

import concourse.bass as bass
import concourse.tile as tile
from concourse import bacc, mybir
from concourse.bass_utils import run_bass_kernel_spmd
from concourse.masks import make_identity

P = 128
C = 256            # channels
N = 4096           # tokens per batch element (h*w)
H = 2048           # query rows per core (half of N)
CT = C // P        # 2 c-tiles
G = 32             # groups
GS = C // G        # 8 channels per group
GPT = P // GS      # 16 groups per c-tile
EPS = 1e-5
QSCALE = C ** -0.5
IBLK = 256         # query i-block
NIB = H // IBLK    # 8
JT = N // P        # 32 key j-chunks
SJ = 4             # j-chunks per scores psum tile / ST tile
NG = JT // SJ      # 8 ST groups per i-block
F32 = mybir.dt.float32
BF16 = mybir.dt.bfloat16

_BUILD_CACHE = {}


def _build_nc():
    nc = bacc.Bacc()
    x_full = nc.declare_dram_parameter("x_full", [C, N], F32, isOutput=False)
    x_half = nc.declare_dram_parameter("x_half", [C, H], F32, isOutput=False)
    gn_gamma = nc.declare_dram_parameter("gn_gamma", [C], F32, isOutput=False)
    gn_beta = nc.declare_dram_parameter("gn_beta", [C], F32, isOutput=False)
    # weights come in pre-transposed ([in, out] layout) and bf16-cast on host
    w_qkvT = nc.declare_dram_parameter("w_qkvT", [C, 3 * C], BF16, isOutput=False)
    b_qkv = nc.declare_dram_parameter("b_qkv", [3 * C], F32, isOutput=False)
    w_outT = nc.declare_dram_parameter("w_outT", [C, C], BF16, isOutput=False)
    b_out = nc.declare_dram_parameter("b_out", [C], F32, isOutput=False)
    out_ext = nc.declare_dram_parameter("out", [C, H], F32, isOutput=True)

    with tile.TileContext(nc) as tc:
        with (
            tc.tile_pool(name="consts", bufs=1) as consts,
            tc.tile_pool(name="acts", bufs=1) as acts,
            tc.tile_pool(name="stp", bufs=2 * NG) as stp,
            tc.tile_pool(name="smalls", bufs=2) as smalls,
            tc.tile_pool(name="stats", bufs=1) as stats_pool,
            tc.tile_pool(name="psS", bufs=2, space="PSUM") as psS,
            tc.tile_pool(name="psV", bufs=2, space="PSUM") as psV,
            tc.tile_pool(name="psT", bufs=2, space="PSUM") as psT,
        ):
            # ---------------- loads ----------------
            xf = acts.tile([P, CT, N], F32)
            for t in range(CT):
                nc.sync.dma_start(
                    out=xf[:, t, :],
                    in_=x_full[:].rearrange("(t p) n -> t p n", p=P)[t],
                )
            xh = acts.tile([P, CT, H], F32)
            nc.sync.dma_start(out=xh, in_=x_half[:].rearrange("(t p) n -> p t n", p=P))

            wqkvT = consts.tile([P, CT, 3 * C], BF16)
            nc.sync.dma_start(
                out=wqkvT, in_=w_qkvT[:].rearrange("(t p) o -> p t o", p=P)
            )
            woT = consts.tile([P, CT, C], BF16)
            nc.sync.dma_start(
                out=woT, in_=w_outT[:].rearrange("(t p) o -> p t o", p=P)
            )
            gamma_p = consts.tile([P, CT], F32)
            nc.sync.dma_start(out=gamma_p, in_=gn_gamma[:].rearrange("(t p) -> p t", p=P))
            beta_p = consts.tile([P, CT], F32)
            nc.sync.dma_start(out=beta_p, in_=gn_beta[:].rearrange("(t p) -> p t", p=P))
            bqkv_p = consts.tile([P, 6], F32)
            nc.sync.dma_start(out=bqkv_p, in_=b_qkv[:].rearrange("(a p) -> p a", p=P))
            bout_p = consts.tile([P, CT], F32)
            nc.sync.dma_start(out=bout_p, in_=b_out[:].rearrange("(t p) -> p t", p=P))
            bv_bc = consts.tile([P, C], F32)
            nc.gpsimd.dma_start(
                out=bv_bc, in_=b_qkv[2 * C : 3 * C][None, :].to_broadcast((P, C))
            )

            ident_b = consts.tile([P, P], BF16)
            make_identity(nc, ident_b)

            # group-aggregation selector: sel[ch, g] = 1/GS if ch//GS == g
            # (built with affine_select: keep where 0 <= ch - GS*g <= GS-1)
            sel = consts.tile([P, GPT], F32)
            nc.gpsimd.memset(sel, 1.0 / GS)
            nc.gpsimd.affine_select(
                out=sel, in_=sel, compare_op=mybir.AluOpType.is_ge, fill=0.0,
                base=0, pattern=[[-GS, GPT]], channel_multiplier=1,
            )
            nc.gpsimd.affine_select(
                out=sel, in_=sel, compare_op=mybir.AluOpType.is_ge, fill=0.0,
                base=GS - 1, pattern=[[GS, GPT]], channel_multiplier=-1,
            )
            # broadcast selector: bsel[g, ch] = 1 if ch//GS == g
            bsel = consts.tile([GPT, P], F32)
            nc.gpsimd.memset(bsel, 1.0)
            nc.gpsimd.affine_select(
                out=bsel, in_=bsel, compare_op=mybir.AluOpType.is_ge, fill=0.0,
                base=0, pattern=[[1, P]], channel_multiplier=-GS,
            )
            nc.gpsimd.affine_select(
                out=bsel, in_=bsel, compare_op=mybir.AluOpType.is_ge, fill=0.0,
                base=GS - 1, pattern=[[-1, P]], channel_multiplier=GS,
            )

            # PE warmup: one matmul whose only dependency is the gpsimd-built
            # constants, so later PE instructions (incl. LDW-fused transposes,
            # which have a single sync-wait slot) never need a fresh gpsimd
            # wait on top of a data wait.
            warm = psT.tile([GPT, GPT], F32, tag="t128")
            nc.tensor.matmul(warm, lhsT=sel, rhs=sel, start=True, stop=True)
            warm2 = psT.tile([P, P], F32, tag="t128")
            nc.tensor.matmul(warm2, lhsT=bsel, rhs=bsel, start=True, stop=True)

            # ---------------- GroupNorm statistics ----------------
            # per-channel mean/var over the 4096 spatial positions
            mv = stats_pool.tile([P, CT, 2], F32)
            for t in range(CT):
                bstats = stats_pool.tile([P, 8, 6], F32, tag="bstats")
                for s in range(8):
                    nc.vector.bn_stats(
                        out=bstats[:, s, :], in_=xf[:, t, s * 512 : (s + 1) * 512]
                    )
                nc.vector.bn_aggr(out=mv[:, t, :], in_=bstats)

            # ts2: col0 = mean_c, col1 = E[x^2]_c = var_c + mean_c^2
            ts2 = stats_pool.tile([P, CT, 2], F32)
            for t in range(CT):
                nc.vector.tensor_copy(out=ts2[:, t, 0:1], in_=mv[:, t, 0:1])
                nc.vector.tensor_mul(ts2[:, t, 1:2], mv[:, t, 0:1], mv[:, t, 0:1])
                nc.vector.tensor_add(ts2[:, t, 1:2], ts2[:, t, 1:2], mv[:, t, 1:2])

            # aggregate channels -> groups:  gv[g] = (M_g, E2_g)
            gv = stats_pool.tile([GPT, CT, 2], F32)
            for t in range(CT):
                gp = psT.tile([GPT, 2], F32, tag="t128")
                nc.tensor.matmul(gp, lhsT=sel, rhs=ts2[:, t, :], start=True, stop=True)
                nc.vector.tensor_copy(out=gv[:, t, :], in_=gp)

            # rstd_g = 1/sqrt(E2 - M^2 + eps)
            eps16 = stats_pool.tile([GPT, 1], F32)
            nc.vector.memset(eps16, EPS)
            gAB = stats_pool.tile([GPT, CT, 2], F32)  # col0 = M_g, col1 = rstd_g
            for t in range(CT):
                msq = stats_pool.tile([GPT, 1], F32, tag="msq")
                nc.vector.tensor_mul(msq, gv[:, t, 0:1], gv[:, t, 0:1])
                var = stats_pool.tile([GPT, 1], F32, tag="var")
                nc.vector.tensor_tensor(
                    out=var, in0=gv[:, t, 1:2], in1=msq, op=mybir.AluOpType.subtract
                )
                sd = stats_pool.tile([GPT, 1], F32, tag="sd")
                nc.scalar.activation(
                    out=sd, in_=var, func=mybir.ActivationFunctionType.Sqrt,
                    bias=eps16, scale=1.0,
                )
                nc.vector.tensor_copy(out=gAB[:, t, 0:1], in_=gv[:, t, 0:1])
                nc.vector.reciprocal(out=gAB[:, t, 1:2], in_=sd)

            # broadcast groups -> channels; per-channel scale/shift
            scale_sb = stats_pool.tile([P, CT, 1], F32)
            shift_sb = stats_pool.tile([P, CT, 1], F32)
            for t in range(CT):
                bp = psT.tile([P, 2], F32, tag="t128")
                nc.tensor.matmul(bp, lhsT=bsel, rhs=gAB[:, t, :], start=True, stop=True)
                chMR = stats_pool.tile([P, 2], F32, tag="chMR")
                nc.vector.tensor_copy(out=chMR, in_=bp)
                # scale = gamma * rstd ; shift = beta - mean * scale
                nc.vector.tensor_mul(scale_sb[:, t, :], gamma_p[:, t, None], chMR[:, 1:2])
                nc.vector.tensor_mul(shift_sb[:, t, :], chMR[:, 0:1], scale_sb[:, t, :])
                nc.vector.tensor_tensor(
                    out=shift_sb[:, t, :], in0=beta_p[:, t, None],
                    in1=shift_sb[:, t, :], op=mybir.AluOpType.subtract,
                )

            # ---------------- apply GN (to bf16) ----------------
            xn = acts.tile([P, CT, N], BF16)
            xnh = acts.tile([P, CT, H], BF16)
            for t in range(CT):
                nc.vector.tensor_scalar(
                    out=xn[:, t, :], in0=xf[:, t, :],
                    scalar1=scale_sb[:, t, :], scalar2=shift_sb[:, t, :],
                    op0=mybir.AluOpType.mult, op1=mybir.AluOpType.add,
                )
                nc.vector.tensor_scalar(
                    out=xnh[:, t, :], in0=xh[:, t, :],
                    scalar1=scale_sb[:, t, :], scalar2=shift_sb[:, t, :],
                    op0=mybir.AluOpType.mult, op1=mybir.AluOpType.add,
                )

            # ---------------- QKV projections ----------------
            # Q[c, i] (scaled by 1/sqrt(C)), from the half rows
            q_sb = acts.tile([P, CT, H], BF16)
            for ot in range(CT):
                for ib in range(H // 512):
                    qp = psS.tile([P, 512], F32, tag="s")
                    for t in range(CT):
                        nc.tensor.matmul(
                            qp,
                            lhsT=wqkvT[:, t, ot * P : (ot + 1) * P],
                            rhs=xnh[:, t, ib * 512 : (ib + 1) * 512],
                            start=(t == 0), stop=(t == CT - 1),
                        )
                    nc.vector.tensor_scalar(
                        out=q_sb[:, ot, ib * 512 : (ib + 1) * 512], in0=qp,
                        scalar1=bqkv_p[:, ot, None], scalar2=float(QSCALE),
                        op0=mybir.AluOpType.add, op1=mybir.AluOpType.mult,
                    )
            # K[c, j] over all rows
            k_sb = acts.tile([P, CT, N], BF16)
            for ot in range(CT):
                for jb in range(N // 512):
                    kp = psS.tile([P, 512], F32, tag="s")
                    for t in range(CT):
                        nc.tensor.matmul(
                            kp,
                            lhsT=wqkvT[:, t, C + ot * P : C + (ot + 1) * P],
                            rhs=xn[:, t, jb * 512 : (jb + 1) * 512],
                            start=(t == 0), stop=(t == CT - 1),
                        )
                    nc.vector.tensor_scalar(
                        out=k_sb[:, ot, jb * 512 : (jb + 1) * 512], in0=kp,
                        scalar1=bqkv_p[:, 2 + ot, None], scalar2=None,
                        op0=mybir.AluOpType.add,
                    )
            # V^T[j, c] plus a ones column for softmax row sums
            v_sb = acts.tile([P, JT, C + 1], BF16)
            nc.gpsimd.memset(v_sb[:, :, C : C + 1], 1.0)
            for jt in range(JT):
                vp = psV.tile([P, C + 1], F32, tag="v")
                for t in range(CT):
                    nc.tensor.matmul(
                        vp[:, :C],
                        lhsT=xn[:, t, jt * P : (jt + 1) * P],
                        rhs=wqkvT[:, t, 2 * C : 3 * C],
                        start=(t == 0), stop=(t == CT - 1),
                    )
                nc.vector.tensor_tensor(
                    out=v_sb[:, jt, :C], in0=vp[:, :C],
                    in1=bv_bc, op=mybir.AluOpType.add,
                )

            # ---------------- attention + output projection ----------------
            out_r = out_ext[:].rearrange("(t p) n -> p t n", p=P)
            for ib in range(NIB):
                i0 = ib * IBLK
                # S^T = K^T Q for this i-block, exp()'d into bf16 ST tiles
                st_tiles = []
                for gidx in range(NG):
                    sp = psS.tile([P, SJ, IBLK], F32, tag="s")
                    for a in range(SJ):
                        jt = gidx * SJ + a
                        for t in range(CT):
                            nc.tensor.matmul(
                                sp[:, a, :],
                                lhsT=k_sb[:, t, jt * P : (jt + 1) * P],
                                rhs=q_sb[:, t, i0 : i0 + IBLK],
                                start=(t == 0), stop=(t == CT - 1),
                            )
                    st = stp.tile([P, SJ, IBLK], BF16, tag="st")
                    nc.scalar.activation(
                        out=st, in_=sp, func=mybir.ActivationFunctionType.Exp
                    )
                    st_tiles.append(st)
                # PV: ao[i, c] with trailing rowsum column, then normalize
                aoT = smalls.tile([P, CT, IBLK], BF16, tag="aoT")
                for isub in range(IBLK // P):
                    pv = psV.tile([P, C + 1], F32, tag="v")
                    for gidx in range(NG):
                        for a in range(SJ):
                            jt = gidx * SJ + a
                            nc.tensor.matmul(
                                pv,
                                lhsT=st_tiles[gidx][:, a, isub * P : (isub + 1) * P],
                                rhs=v_sb[:, jt, :],
                                start=(jt == 0), stop=(jt == JT - 1),
                            )
                    rsum = smalls.tile([P, 1], F32, tag="rsum")
                    nc.vector.reciprocal(out=rsum, in_=pv[:, C : C + 1])
                    ao = smalls.tile([P, C], BF16, tag="ao")
                    nc.vector.tensor_scalar(
                        out=ao, in0=pv[:, :C], scalar1=rsum, scalar2=None,
                        op0=mybir.AluOpType.mult,
                    )
                    # transpose ao -> aoT[c, i]
                    for t in range(CT):
                        tp = psT.tile([P, P], BF16, tag="t128")
                        nc.tensor.transpose(tp, ao[:, t * P : (t + 1) * P], ident_b)
                        nc.vector.tensor_copy(
                            out=aoT[:, t, isub * P : (isub + 1) * P], in_=tp
                        )
                # output projection + bias + residual
                out_sb = smalls.tile([P, CT, IBLK], F32, tag="out_sb")
                for ot in range(CT):
                    op = psV.tile([P, C + 1], F32, tag="v")
                    for t in range(CT):
                        nc.tensor.matmul(
                            op[:, :IBLK],
                            lhsT=woT[:, t, ot * P : (ot + 1) * P],
                            rhs=aoT[:, t, :],
                            start=(t == 0), stop=(t == CT - 1),
                        )
                    nc.vector.tensor_scalar(
                        out=out_sb[:, ot, :], in0=op[:, :IBLK],
                        scalar1=bout_p[:, ot, None], scalar2=None,
                        op0=mybir.AluOpType.add,
                    )
                    nc.vector.tensor_add(
                        out_sb[:, ot, :], out_sb[:, ot, :], xh[:, ot, i0 : i0 + IBLK]
                    )
                nc.sync.dma_start(out=out_r[:, :, i0 : i0 + IBLK], in_=out_sb)

    nc.finalize()
    return nc


def kernel(x, gn_gamma, gn_beta, w_qkv, b_qkv, w_out, b_out, _trace=False):
    import kernel as _self

    b, c, h, w = x.shape
    assert (b, c, h, w) == (4, 256, 64, 64)
    x = np.ascontiguousarray(np.asarray(x, dtype=np.float32))

    if "nc" not in _BUILD_CACHE:
        _BUILD_CACHE["nc"] = _build_nc()
    nc = _BUILD_CACHE["nc"]

    import ml_dtypes

    w_qkvT = np.ascontiguousarray(
        np.asarray(w_qkv, np.float32).T.astype(ml_dtypes.bfloat16)
    )
    w_outT = np.ascontiguousarray(
        np.asarray(w_out, np.float32).T.astype(ml_dtypes.bfloat16)
    )
    in_maps = []
    for core in range(8):
        bi, hi = core // 2, core % 2
        in_maps.append(
            {
                "x_full": x[bi].reshape(C, N),
                "x_half": np.ascontiguousarray(
                    x[bi, :, 32 * hi : 32 * hi + 32, :]
                ).reshape(C, H),
                "gn_gamma": np.asarray(gn_gamma, np.float32),
                "gn_beta": np.asarray(gn_beta, np.float32),
                "w_qkvT": w_qkvT,
                "b_qkv": np.asarray(b_qkv, np.float32),
                "w_outT": w_outT,
                "b_out": np.asarray(b_out, np.float32),
            }
        )

    res = run_bass_kernel_spmd(nc, in_maps, core_ids=list(range(8)), trace=_trace)
    _self._LAST_RESULT = res

    out = np.empty((b, c, h, w), dtype=np.float32)
    for core in range(8):
        bi, hi = core // 2, core % 2
        out[bi, :, 32 * hi : 32 * hi + 32, :] = res.results[core]["out"].reshape(
            C, 32, 64
        )
    return out
